# revision 1
# baseline (speedup 1.0000x reference)
"""Gated spiking reservoir step — Trainium2 Bass kernel (8 NeuronCores).

Math (per reference):
    ic   = inputs @ input_weights                  # [B, R]
    rc   = reservoir_state @ reservoir_weights     # [B, R]
    gate = sigmoid(inputs @ gate_weights)          # [B, R]
    ns   = (0.9 * reservoir_state + 0.1 * tanh(ic + rc)) * gate
    out  = (ns > 0.5) ? 1.0 : 0.0
    returns (out, ns)

Sharding: tensor-parallel over the reservoir (output-column) dim.  Each of
the 8 cores owns a 512-column slice of all three weight matrices and
produces the matching [512-column x full-batch] slice of both outputs.
The activations (inputs / reservoir_state) are replicated, pre-transposed
on host to [K, B] so the contraction dim lands on SBUF partitions.

On-device layout per core (everything transposed — state tiles are
[cols(part) x batch(free)]):
    for each batch slice of 512:
        gate_ps[c]  += w_gate[k-tile, c-tile].T @ xT[k-tile, b-slice]   (8 k-tiles)
        state_ps[c] += w_in  [k-tile, c-tile].T @ xT[k-tile, b-slice]   (8 k-tiles)
        state_ps[c] += w_res [k-tile, c-tile].T @ sT[k-tile, b-slice]   (32 k-tiles)
        t  = tanh(state_ps[c])        (ScalarE, from PSUM)
        g  = sigmoid(gate_ps[c])      (ScalarE, from PSUM)
        v  = (s_slice * 9.0) + t      (VectorE scalar_tensor_tensor)
        ns = (v * 0.1) * g            (VectorE scalar_tensor_tensor)
        spk = ns > 0.5                (VectorE tensor_scalar is_gt -> 1.0/0.0)
"""

import os
import sys

if "/opt/trn_rl_repo" not in sys.path:
    sys.path.insert(0, "/opt/trn_rl_repo")

import numpy as np

B, D_IN, R = 2048, 1024, 4096
N_CORES = 8
COLS = R // N_CORES          # 512 output columns per core
P = 128                      # SBUF/PSUM partitions
NB = 512                     # batch free-dim per matmul / PSUM bank
KI = D_IN // P               # 8 k-tiles over the input dim
KR = R // P                  # 32 k-tiles over the reservoir dim
CT = COLS // P               # 4 column tiles per core
BT = B // NB                 # 4 batch slices

# float32r runs the PE at 4x the fp32 rate for moving dim >= 256.
MM_DTYPE = os.environ.get("BASS_MM_DTYPE", "float32r")

_CACHE = {}


def _build(mm_dtype_name: str):
    from contextlib import ExitStack

    from concourse import bacc, tile
    import concourse.mybir as mybir

    f32 = mybir.dt.float32
    mm_dt = getattr(mybir.dt, mm_dtype_name)
    AF = mybir.ActivationFunctionType
    ALU = mybir.AluOpType

    nc = bacc.Bacc(
        "TRN2", target_bir_lowering=False, debug=False, enable_asserts=False
    )

    xT = nc.dram_tensor("xT", [D_IN, B], f32, kind="ExternalInput")
    sT = nc.dram_tensor("sT", [R, B], f32, kind="ExternalInput")
    w_in = nc.dram_tensor("w_in", [D_IN, COLS], f32, kind="ExternalInput")
    w_res = nc.dram_tensor("w_res", [R, COLS], f32, kind="ExternalInput")
    w_gate = nc.dram_tensor("w_gate", [D_IN, COLS], f32, kind="ExternalInput")
    nsT = nc.dram_tensor("nsT", [COLS, B], f32, kind="ExternalOutput")
    spkT = nc.dram_tensor("spkT", [COLS, B], mybir.dt.uint8, kind="ExternalOutput")

    def cast(ap):
        return ap.bitcast(mm_dt) if mm_dtype_name != "float32" else ap

    with tile.TileContext(nc) as tc, ExitStack() as ctx:
        # Resident weights: 12 MB of SBUF (96 KB/partition), one tile per
        # 128-row k-slice so matmuls only wait on the slice they consume.
        wpool = ctx.enter_context(tc.tile_pool(name="weights", bufs=1))
        w_in_sb, w_gate_sb, w_res_sb = [], [], []
        for k in range(KI):
            t = wpool.tile([P, COLS], mm_dt, tag=f"w_in_{k}", name=f"w_in_sb{k}")
            w_in_sb.append(t)
            t = wpool.tile([P, COLS], mm_dt, tag=f"w_gate_{k}", name=f"w_gate_sb{k}")
            w_gate_sb.append(t)
        for k in range(KR):
            t = wpool.tile([P, COLS], mm_dt, tag=f"w_res_{k}", name=f"w_res_sb{k}")
            w_res_sb.append(t)

        # All weights are JIT-issued on HWDGE right before first use
        # (w_in/w_gate in slice 0's x-phase, w_res in slice 0's s-phase).
        def load_w_res(k):
            nc.sync.dma_start(w_res_sb[k][:], cast(w_res[k * P : (k + 1) * P, :]))

        xpool = ctx.enter_context(tc.tile_pool(name="x_mov", bufs=6))
        spool = ctx.enter_context(tc.tile_pool(name="s_mov", bufs=6))
        st_psum = ctx.enter_context(tc.tile_pool(name="st_ps", bufs=4, space="PSUM"))
        gt_psum = ctx.enter_context(tc.tile_pool(name="gt_ps", bufs=4, space="PSUM"))
        epool = ctx.enter_context(tc.tile_pool(name="epilogue", bufs=3))

        for b in range(BT):
            bs = slice(b * NB, (b + 1) * NB)
            state_ps = [st_psum.tile([P, NB], f32, tag="state", name=f"state_ps_{b}_{i}") for i in range(CT)]
            gate_ps = [gt_psum.tile([P, NB], f32, tag="gate", name=f"gate_ps_{b}_{i}") for i in range(CT)]

            # Gate matmuls first so gate PSUM banks retire early.
            for k in range(KI):
                xt = xpool.tile([P, NB], mm_dt, tag="xt")
                nc.sync.dma_start(xt[:], cast(xT[k * P : (k + 1) * P, bs]))
                if b == 0:
                    nc.sync.dma_start(
                        w_gate_sb[k][:], cast(w_gate[k * P : (k + 1) * P, :])
                    )
                    nc.sync.dma_start(
                        w_in_sb[k][:], cast(w_in[k * P : (k + 1) * P, :])
                    )
                for c in range(CT):
                    nc.tensor.matmul(
                        gate_ps[c][:],
                        w_gate_sb[k][:, c * P : (c + 1) * P],
                        xt[:],
                        start=(k == 0),
                        stop=(k == KI - 1),
                    )
                for c in range(CT):
                    nc.tensor.matmul(
                        state_ps[c][:],
                        w_in_sb[k][:, c * P : (c + 1) * P],
                        xt[:],
                        start=(k == 0),
                        stop=False,
                    )
            # s-phase A: first half of the k-tiles, all column tiles in
            # lockstep (k-major) so each st tile is short-lived.
            KH = KR // 2
            st_ep = []
            for k in range(KH):
                if b == 0:
                    load_w_res(k)
                if k < CT:
                    # This core's own state rows (epilogue reads them too):
                    # keep an exact fp32 copy, round to f32r on-chip for PE.
                    sf = spool.tile([P, NB], f32, tag="stEp", bufs=9,
                                    name=f"stEp_{b}_{k}")
                    nc.sync.dma_start(sf[:], sT[k * P : (k + 1) * P, bs])
                    st_ep.append(sf)
                    if mm_dtype_name != "float32":
                        st = spool.tile([P, NB], mm_dt, tag="stEpR", bufs=3,
                                        name=f"stEpR_{b}_{k}")
                        nc.scalar.copy(st[:], sf[:])
                    else:
                        st = sf
                else:
                    st = spool.tile([P, NB], mm_dt, tag="st")
                    nc.sync.dma_start(st[:], cast(sT[k * P : (k + 1) * P, bs]))
                for c in range(CT):
                    nc.tensor.matmul(
                        state_ps[c][:],
                        w_res_sb[k][:, c * P : (c + 1) * P],
                        st[:],
                        start=False,
                        stop=False,
                    )
            # s-phase B: second half column-major, so state_ps[c] finishes
            # (and its PSUM slot frees via tanh) staggered well before the
            # slice ends -- removes the PE bubble at slice boundaries.
            stB = []
            for k in range(KH, KR):
                if b == 0:
                    load_w_res(k)
                st = spool.tile([P, NB], mm_dt, tag="stB", bufs=18, name=f"stB_{b}_{k}")
                nc.sync.dma_start(st[:], cast(sT[k * P : (k + 1) * P, bs]))
                stB.append(st)
            for c in range(CT):
                for j, k in enumerate(range(KH, KR)):
                    nc.tensor.matmul(
                        state_ps[c][:],
                        w_res_sb[k][:, c * P : (c + 1) * P],
                        stB[j][:],
                        start=False,
                        stop=(k == KR - 1),
                    )

            NH = NB // 2
            for c in range(CT):
                cs = slice(c * P, (c + 1) * P)
                se_f32 = st_ep[c]
                ns = epool.tile([P, NB], f32, tag="ns", name=f"ns_{b}_{c}")
                spk = epool.tile([P, NB], mybir.dt.uint8, tag="spk",
                                 name=f"spk_{b}_{c}")
                for h in range(2):
                    hs = slice(h * NH, (h + 1) * NH)
                    tt = epool.tile([P, NH], f32, tag="tanh")
                    nc.scalar.activation(tt[:], state_ps[c][:, hs], AF.Tanh)
                    gg = epool.tile([P, NH], f32, tag="sig")
                    nc.scalar.activation(gg[:], gate_ps[c][:, hs], AF.Sigmoid)
                    vv = epool.tile([P, NH], f32, tag="v")
                    nc.vector.scalar_tensor_tensor(
                        vv[:], se_f32[:, hs], 9.0, tt[:], ALU.mult, ALU.add
                    )
                    nc.vector.scalar_tensor_tensor(
                        ns[:, hs], vv[:], 0.1, gg[:], ALU.mult, ALU.mult
                    )
                    nc.vector.tensor_scalar(
                        spk[:, hs], ns[:, hs], 0.5, None, ALU.is_gt
                    )
                nc.sync.dma_start(nsT[cs, bs], ns[:])
                nc.sync.dma_start(spkT[cs, bs], spk[:])

    nc.compile()
    return nc


def _get_program():
    if MM_DTYPE not in _CACHE:
        _CACHE[MM_DTYPE] = _build(MM_DTYPE)
    return _CACHE[MM_DTYPE]


def kernel(inputs, prev_output, reservoir_state, input_weights, reservoir_weights,
           gate_weights):
    from concourse.bass_utils import run_bass_kernel_spmd

    nc = _get_program()

    x = np.ascontiguousarray(np.asarray(inputs, dtype=np.float32))
    s = np.ascontiguousarray(np.asarray(reservoir_state, dtype=np.float32))
    w_in = np.ascontiguousarray(np.asarray(input_weights, dtype=np.float32))
    w_res = np.ascontiguousarray(np.asarray(reservoir_weights, dtype=np.float32))
    w_gate = np.ascontiguousarray(np.asarray(gate_weights, dtype=np.float32))

    xT = np.ascontiguousarray(x.T)          # [D_IN, B]
    sT = np.ascontiguousarray(s.T)          # [R, B]

    in_maps = []
    for core in range(N_CORES):
        c0 = core * COLS
        cs = slice(c0, c0 + COLS)
        # Rotate the contraction (reservoir-row) order so this core's own
        # 512 state rows arrive as k-tiles 0..3 -- the epilogue reuses those
        # SBUF tiles directly instead of re-reading them from HBM.  The same
        # rotation is applied to w_res rows, so the dot products are
        # unchanged (summation is commutative).
        w_res_c = w_res[:, cs]
        in_maps.append(
            {
                "xT": xT,
                "sT": np.concatenate([sT[c0:], sT[:c0]], axis=0),
                "w_in": np.ascontiguousarray(w_in[:, cs]),
                "w_res": np.concatenate([w_res_c[c0:], w_res_c[:c0]], axis=0),
                "w_gate": np.ascontiguousarray(w_gate[:, cs]),
            }
        )

    res = run_bass_kernel_spmd(nc, in_maps, list(range(N_CORES)))

    ns_T = np.concatenate([res.results[c]["nsT"] for c in range(N_CORES)], axis=0)
    spk_T = np.concatenate([res.results[c]["spkT"] for c in range(N_CORES)], axis=0)
    new_state = np.ascontiguousarray(ns_T.T)     # [B, R]
    output = spk_T.T.astype(np.float32)          # [B, R]

    # The PE's fast fp32 path (float32r) rounds matmul operands to ~12
    # mantissa bits, so new_state carries ~3e-4 absolute error.  That only
    # matters for the binary spike output where new_state sits within that
    # error of the 0.5 threshold.  Re-evaluate just those borderline
    # elements (~0.1% of the tensor) at full precision and patch both
    # outputs, restoring plain-fp32-level accuracy for the thresholding.
    bi, rj = np.nonzero(np.abs(new_state - 0.5) < 1.5e-3)
    if bi.size:
        xg = x[bi].astype(np.float64)
        sg = s[bi].astype(np.float64)
        acc = np.einsum("ij,ji->i", xg, w_in[:, rj], optimize=True)
        acc += np.einsum("ij,ji->i", sg, w_res[:, rj], optimize=True)
        gate = 1.0 / (1.0 + np.exp(-np.einsum("ij,ji->i", xg, w_gate[:, rj],
                                              optimize=True)))
        ns_fix = (0.9 * s[bi, rj].astype(np.float64)
                  + 0.1 * np.tanh(acc)) * gate
        ns_fix32 = ns_fix.astype(np.float32)
        new_state[bi, rj] = ns_fix32
        output[bi, rj] = (ns_fix32 > 0.5).astype(np.float32)
    return output, new_state



# revision 3
# speedup vs baseline: 3.0494x; 3.0494x over previous
"""Gated spiking reservoir step — Trainium2 Bass kernel (8 NeuronCores).

Math (per reference):
    ic   = inputs @ input_weights                  # [B, R]
    rc   = reservoir_state @ reservoir_weights     # [B, R]
    gate = sigmoid(inputs @ gate_weights)          # [B, R]
    ns   = (0.9 * reservoir_state + 0.1 * tanh(ic + rc)) * gate
    out  = (ns > 0.5) ? 1.0 : 0.0
    returns (out, ns)

Strategy
--------
2-D sharding: 4 column groups x 2 batch groups.  Each core owns a
[1024-column x 1024-batch] block of the output and loads the matching
weight column slice plus the activation batch slice (transposed to [K, B]
so the contraction dim lands on SBUF partitions).

All three GEMMs run in fp8 (e4m3) with MatmulPerfMode.DoubleRow: each
matmul instruction contracts K=256 (two 128-row sub-tiles) at 0.5
cycles/row -- 4x the fp32r/bf16 rate.  Weights are pre-scaled by 64 on
host so their 0.02-std distribution lands in e4m3's normal range; the
64x is divided back out by the activation-function input scale.

Epilogue (per [128 x 512] output tile):
    t  = bf16(tanh(state_psum / 64))       ScalarE, PSUM -> SBUF
    g  = bf16(sigmoid(gate_psum / 64))     ScalarE
    v  = t + sw9                           VectorE tensor_tensor (bf16, 2x)
    o  = v * g                             VectorE tensor_tensor (bf16, 2x)
where sw9 = bf16(9 * s) is host-prepared, so the device returns
o = (9s + tanh(.)) * sigmoid(.) = 10 * new_state.  The host multiplies
by 0.1, thresholds at 0.5 for the spike output, and re-evaluates the
few borderline elements (|ns - 0.5| < band) in full precision to absorb
the fp8 quantization error at the spike threshold.
"""

import os
import sys

if "/opt/trn_rl_repo" not in sys.path:
    sys.path.insert(0, "/opt/trn_rl_repo")

import numpy as np
import ml_dtypes

B, D_IN, R = 2048, 1024, 4096
N_CORES = 8
CGRP, BGRP = 4, 2            # column groups x batch groups
C = R // CGRP                # 1024 output columns per core
BS = B // BGRP               # 1024 batch rows per core
P = 128                      # SBUF/PSUM partitions
NB = 512                     # moving free dim per matmul / PSUM bank
SL = BS // NB                # 2 batch slices per core
KB_I = D_IN // 256           # 4  fp8 DoubleRow k-blocks over the input dim
KB_R = R // 256              # 16 fp8 DoubleRow k-blocks over the reservoir dim
KT_G = D_IN // P             # 8  bf16 k-tiles over the input dim (gate, bf16 mode)
CT = C // P                  # 8 column tiles per core
WSCALE = 64.0                # host pre-scale for fp8 weights

# "fp8": gate GEMM also fp8 DoubleRow (fastest; ns rel-err ~1.2e-2)
# "bf16": gate GEMM in bf16 (ns rel-err ~5e-3, +20us)
GATE_MODE = os.environ.get("BASS_GATE_MODE", "fp8")
PATCH_BAND = float(os.environ.get("BASS_PATCH_BAND", "0.04" if GATE_MODE == "fp8" else "0.02"))

F8NP = ml_dtypes.float8_e4m3
BFNP = ml_dtypes.bfloat16

_CACHE = {}


def _build(gate_mode: str):
    from contextlib import ExitStack

    from concourse import bacc, tile
    import concourse.mybir as mybir

    f32 = mybir.dt.float32
    f8 = mybir.dt.float8e4
    bf = mybir.dt.bfloat16
    AF = mybir.ActivationFunctionType
    DR = mybir.MatmulPerfMode.DoubleRow
    ALU = mybir.AluOpType

    nc = bacc.Bacc(
        "TRN2", target_bir_lowering=False, debug=False, enable_asserts=False
    )

    x8d = nc.dram_tensor("x8", [D_IN, BS], f8, kind="ExternalInput")
    s8d = nc.dram_tensor("s8", [R, BS], f8, kind="ExternalInput")
    sw9d = nc.dram_tensor("sw9", [C, BS], bf, kind="ExternalInput")
    wi8d = nc.dram_tensor("wi8", [D_IN, C], f8, kind="ExternalInput")
    wr8d = nc.dram_tensor("wr8", [R, C], f8, kind="ExternalInput")
    if gate_mode == "fp8":
        wg8d = nc.dram_tensor("wg8", [D_IN, C], f8, kind="ExternalInput")
    else:
        wgbd = nc.dram_tensor("wgb", [D_IN, C], bf, kind="ExternalInput")
        xbd = nc.dram_tensor("xb", [D_IN, BS], bf, kind="ExternalInput")
    nspd = nc.dram_tensor("nsp", [C, BS], bf, kind="ExternalOutput")

    def dr3(dram, kb, cols):
        # [256 x n] HBM block -> [128, 2, n] (partition, k-sub-row, free)
        return dram[kb * 256 : (kb + 1) * 256, cols].rearrange(
            "(s p) b -> p s b", p=P
        )

    with tile.TileContext(nc) as tc, ExitStack() as ctx:
        wp = ctx.enter_context(tc.tile_pool(name="w", bufs=1))
        mp = ctx.enter_context(tc.tile_pool(name="m", bufs=1))
        epool = ctx.enter_context(tc.tile_pool(name="e", bufs=1))
        pp = ctx.enter_context(tc.tile_pool(name="ps", bufs=8, space="PSUM"))

        wi_sb = [wp.tile([P, 2, C], f8, tag=f"wi{k}", name=f"wi_sb{k}")
                 for k in range(KB_I)]
        wr_sb = [wp.tile([P, 2, C], f8, tag=f"wr{k}", name=f"wr_sb{k}")
                 for k in range(KB_R)]
        if gate_mode == "fp8":
            wg_sb = [wp.tile([P, 2, C], f8, tag=f"wg{k}", name=f"wg_sb{k}")
                     for k in range(KB_I)]
        else:
            wg_sb = [wp.tile([P, C], bf, tag=f"wg{k}", name=f"wg_sb{k}")
                     for k in range(KT_G)]

        xm = [[mp.tile([P, 2, NB], f8, tag=f"x{sl}_{k}", name=f"xm{sl}_{k}")
               for k in range(KB_I)] for sl in range(SL)]
        sm = [[mp.tile([P, 2, NB], f8, tag=f"s{sl}_{k}", name=f"sm{sl}_{k}")
               for k in range(KB_R)] for sl in range(SL)]
        if gate_mode != "fp8":
            xbm = [[mp.tile([P, NB], bf, tag=f"xb{sl}_{k}", name=f"xbm{sl}_{k}")
                    for k in range(KT_G)] for sl in range(SL)]
        sw_sb = [mp.tile([P, CT, NB], bf, tag=f"sw{sl}", name=f"sw_sb{sl}")
                 for sl in range(SL)]

        t_sb = [[epool.tile([P, NB], bf, tag=f"t{sl}_{c}", name=f"t{sl}_{c}")
                 for c in range(CT)] for sl in range(SL)]
        g_sb = [[epool.tile([P, NB], bf, tag=f"g{sl}_{c}", name=f"g{sl}_{c}")
                 for c in range(CT)] for sl in range(SL)]
        v_sb = [[epool.tile([P, NB], bf, tag=f"v{sl}_{c}", name=f"v{sl}_{c}")
                 for c in range(CT)] for sl in range(SL)]
        ns_sb = [[epool.tile([P, 2, NB], bf, tag=f"ns{sl}_{gidx}",
                             name=f"ns{sl}_{gidx}")
                  for gidx in range(CT // 2)] for sl in range(SL)]

        def bsl(sl):
            return slice(sl * NB, (sl + 1) * NB)

        # ---- input DMA stream (SP queue), in consumption order ----
        for k in range(KB_I):
            nc.sync.dma_start(wi_sb[k][:], dr3(wi8d, k, slice(0, C)))
            nc.sync.dma_start(xm[0][k][:], dr3(x8d, k, bsl(0)))
        for k in range(KB_R):
            nc.sync.dma_start(wr_sb[k][:], dr3(wr8d, k, slice(0, C)))
            nc.sync.dma_start(sm[0][k][:], dr3(s8d, k, bsl(0)))
        if gate_mode == "fp8":
            for k in range(KB_I):
                nc.sync.dma_start(wg_sb[k][:], dr3(wg8d, k, slice(0, C)))
        else:
            for k in range(KT_G):
                nc.sync.dma_start(wg_sb[k][:], wgbd[k * P : (k + 1) * P, :])
                nc.sync.dma_start(xbm[0][k][:], xbd[k * P : (k + 1) * P, bsl(0)])
        for k in range(KB_I):
            nc.sync.dma_start(xm[1][k][:], dr3(x8d, k, bsl(1)))
        if gate_mode != "fp8":
            for k in range(KT_G):
                nc.sync.dma_start(xbm[1][k][:], xbd[k * P : (k + 1) * P, bsl(1)])
        for k in range(KB_R):
            nc.sync.dma_start(sm[1][k][:], dr3(s8d, k, bsl(1)))
        for sl in range(SL):
            nc.sync.dma_start(
                sw_sb[sl][:],
                sw9d[:, bsl(sl)].rearrange("(c p) b -> p c b", p=P),
            )

        def state_matmuls(st, sl, cts):
            # full K accumulation for the given column tiles of slice sl
            for k in range(KB_I):
                for c in cts:
                    nc.tensor.matmul(
                        st[c][:],
                        wi_sb[k][:, :, c * P : (c + 1) * P],
                        xm[sl][k][:],
                        start=(k == 0),
                        stop=False,
                        perf_mode=DR,
                    )
            for k in range(KB_R):
                for c in cts:
                    nc.tensor.matmul(
                        st[c][:],
                        wr_sb[k][:, :, c * P : (c + 1) * P],
                        sm[sl][k][:],
                        start=False,
                        stop=(k == KB_R - 1),
                        perf_mode=DR,
                    )

        def gate_matmuls(gt, sl, c):
            cs = slice(c * P, (c + 1) * P)
            if gate_mode == "fp8":
                for k in range(KB_I):
                    nc.tensor.matmul(
                        gt[:], wg_sb[k][:, :, cs], xm[sl][k][:],
                        start=(k == 0), stop=(k == KB_I - 1), perf_mode=DR,
                    )
            else:
                for k in range(KT_G):
                    nc.tensor.matmul(
                        gt[:], wg_sb[k][:, cs], xbm[sl][k][:],
                        start=(k == 0), stop=(k == KT_G - 1),
                    )

        gate_scale = 1.0 / WSCALE if gate_mode == "fp8" else 1.0

        def emit_ns(sl, c):
            # o = v * g into the ns staging tile; DMA out after each pair
            nc.vector.tensor_tensor(
                ns_sb[sl][c // 2][:, c % 2, :], v_sb[sl][c][:], g_sb[sl][c][:],
                ALU.mult,
            )
            if c % 2 == 1:
                gidx = c // 2
                nc.sync.dma_start(
                    nspd[gidx * 2 * P : (gidx + 1) * 2 * P, bsl(sl)].rearrange(
                        "(t p) b -> p t b", p=P
                    ),
                    ns_sb[sl][gidx][:],
                )

        # ---- slice 0 state, kb-major (DMA-streamed) ----
        st0 = [pp.tile([P, NB], f32, tag="acc", name=f"st0_{c}")
               for c in range(CT)]
        state_matmuls(st0, 0, range(CT))
        for c in range(CT):
            nc.scalar.activation(t_sb[0][c][:], st0[c][:], AF.Tanh,
                                 scale=1.0 / WSCALE)
        for c in range(CT):
            nc.vector.tensor_tensor(v_sb[0][c][:], t_sb[0][c][:],
                                    sw_sb[0][:, c, :], ALU.add)

        # ---- interleave: gate(slice 0) + state(slice 1), column-major ----
        st1 = [None] * CT
        for c in range(CT):
            gt0_c = pp.tile([P, NB], f32, tag="acc", name=f"gt0_{c}")
            gate_matmuls(gt0_c, 0, c)
            nc.scalar.activation(g_sb[0][c][:], gt0_c[:], AF.Sigmoid,
                                 scale=gate_scale)
            emit_ns(0, c)
            st1[c] = pp.tile([P, NB], f32, tag="acc", name=f"st1_{c}")
            state_matmuls(st1, 1, [c])
            nc.scalar.activation(t_sb[1][c][:], st1[c][:], AF.Tanh,
                                 scale=1.0 / WSCALE)
            nc.vector.tensor_tensor(v_sb[1][c][:], t_sb[1][c][:],
                                    sw_sb[1][:, c, :], ALU.add)

        # ---- slice 1 gate ----
        for c in range(CT):
            gt1_c = pp.tile([P, NB], f32, tag="acc", name=f"gt1_{c}")
            gate_matmuls(gt1_c, 1, c)
            nc.scalar.activation(g_sb[1][c][:], gt1_c[:], AF.Sigmoid,
                                 scale=gate_scale)
            emit_ns(1, c)

    nc.compile()
    return nc


def _get_program():
    if GATE_MODE not in _CACHE:
        _CACHE[GATE_MODE] = _build(GATE_MODE)
    return _CACHE[GATE_MODE]


def kernel(inputs, prev_output, reservoir_state, input_weights,
           reservoir_weights, gate_weights):
    from concourse.bass_utils import run_bass_kernel_spmd

    nc = _get_program()

    x = np.ascontiguousarray(np.asarray(inputs, dtype=np.float32))
    s = np.ascontiguousarray(np.asarray(reservoir_state, dtype=np.float32))
    w_in = np.asarray(input_weights, dtype=np.float32)
    w_res = np.asarray(reservoir_weights, dtype=np.float32)
    w_gate = np.asarray(gate_weights, dtype=np.float32)

    xT = np.ascontiguousarray(x.T)           # [D_IN, B]
    sT = np.ascontiguousarray(s.T)           # [R, B]
    x8 = xT.astype(F8NP)
    s8 = sT.astype(F8NP)
    sw9 = (9.0 * sT).astype(BFNP)            # [R, B]
    wi8 = (w_in * WSCALE).astype(F8NP)       # [D_IN, R]
    wr8 = (w_res * WSCALE).astype(F8NP)      # [R, R]
    if GATE_MODE == "fp8":
        wg8 = (w_gate * WSCALE).astype(F8NP)
    else:
        wgb = w_gate.astype(BFNP)
        xb = xT.astype(BFNP)

    in_maps = []
    for core in range(N_CORES):
        bg, cg = divmod(core, CGRP)
        cs = slice(cg * C, (cg + 1) * C)
        bs_ = slice(bg * BS, (bg + 1) * BS)
        m = {
            "x8": np.ascontiguousarray(x8[:, bs_]),
            "s8": np.ascontiguousarray(s8[:, bs_]),
            "sw9": np.ascontiguousarray(sw9[cs, bs_]),
            "wi8": np.ascontiguousarray(wi8[:, cs]),
            "wr8": np.ascontiguousarray(wr8[:, cs]),
        }
        if GATE_MODE == "fp8":
            m["wg8"] = np.ascontiguousarray(wg8[:, cs])
        else:
            m["wgb"] = np.ascontiguousarray(wgb[:, cs])
            m["xb"] = np.ascontiguousarray(xb[:, bs_])
        in_maps.append(m)

    res = run_bass_kernel_spmd(nc, in_maps, list(range(N_CORES)))

    nsp = np.empty((R, B), dtype=np.float32)  # 10 * new_state, transposed
    for core in range(N_CORES):
        bg, cg = divmod(core, CGRP)
        nsp[cg * C : (cg + 1) * C, bg * BS : (bg + 1) * BS] = (
            res.results[core]["nsp"].astype(np.float32)
        )
    new_state = np.ascontiguousarray(nsp.T) * np.float32(0.1)  # [B, R]

    # Re-evaluate borderline elements (|ns - 0.5| < band) in full precision
    # so fp8/bf16 quantization error cannot flip spikes at the threshold.
    bi, rj = np.nonzero(np.abs(new_state - 0.5) < PATCH_BAND)
    if bi.size:
        CH = 32768
        for lo in range(0, bi.size, CH):
            bc, rc = bi[lo : lo + CH], rj[lo : lo + CH]
            xb_ = x[bc]                       # [n, D_IN]
            sb_ = s[bc]                       # [n, R]
            acc = np.einsum("ij,ji->i", xb_, w_in[:, rc], optimize=True)
            acc += np.einsum("ij,ji->i", sb_, w_res[:, rc], optimize=True)
            z = np.einsum("ij,ji->i", xb_, w_gate[:, rc], optimize=True)
            gate = 1.0 / (1.0 + np.exp(-z.astype(np.float64)))
            ns_fix = (0.9 * sb_[np.arange(bc.size), rc].astype(np.float64)
                      + 0.1 * np.tanh(acc.astype(np.float64))) * gate
            new_state[bc, rc] = ns_fix.astype(np.float32)

    output = (new_state > np.float32(0.5)).astype(np.float32)
    return output, new_state


# revision 35
# speedup vs baseline: 3.5861x; 1.1760x over previous
"""Gated spiking reservoir step — Trainium2 Bass kernel (8 NeuronCores).

Math (per reference):
    ic   = inputs @ input_weights                  # [B, R]
    rc   = reservoir_state @ reservoir_weights     # [B, R]
    gate = sigmoid(inputs @ gate_weights)          # [B, R]
    ns   = (0.9 * reservoir_state + 0.1 * tanh(ic + rc)) * gate
    out  = (ns > 0.5) ? 1.0 : 0.0
    returns (out, ns)

Strategy
--------
2-D sharding: 4 column groups x 2 batch groups.  Each core owns a
[1024-column x 1024-batch] block of the output and loads the matching
weight column slice plus the activation batch slice (transposed to [K, B]
so the contraction dim lands on SBUF partitions).

All three GEMMs run in fp8 (e4m3) with MatmulPerfMode.DoubleRow: each
matmul instruction contracts K=256 (two 128-row sub-tiles) at 0.5
cycles/row -- 4x the fp32r/bf16 rate.  Weights are pre-scaled by 64 on
host so their 0.02-std distribution lands in e4m3's normal range; the
64x is divided back out by the activation-function input scale.

Epilogue (per [128 x 512] output tile):
    t  = bf16(tanh(state_psum / 64))       ScalarE, PSUM -> SBUF
    g  = bf16(sigmoid(gate_psum / 64))     ScalarE
    v  = t + sw9                           VectorE tensor_tensor (bf16, 2x)
    o  = v * g                             VectorE tensor_tensor (bf16, 2x)
where sw9 = bf16(9 * s) is host-prepared, so the device returns
o = (9s + tanh(.)) * sigmoid(.) = 10 * new_state.  The host multiplies
by 0.1, thresholds at 0.5 for the spike output, and re-evaluates the
few borderline elements (|ns - 0.5| < band) in full precision to absorb
the fp8 quantization error at the spike threshold.
"""

import os
import sys

if "/opt/trn_rl_repo" not in sys.path:
    sys.path.insert(0, "/opt/trn_rl_repo")

import numpy as np
import ml_dtypes

B, D_IN, R = 2048, 1024, 4096
N_CORES = 8
CGRP, BGRP = 4, 2            # column groups x batch groups
C = R // CGRP                # 1024 output columns per core
BS = B // BGRP               # 1024 batch rows per core
P = 128                      # SBUF/PSUM partitions
NB = 512                     # moving free dim per matmul / PSUM bank
SL = BS // NB                # 2 batch slices per core
KB_I = D_IN // 256           # 4  fp8 DoubleRow k-blocks over the input dim
KB_R = R // 256              # 16 fp8 DoubleRow k-blocks over the reservoir dim
KT_G = D_IN // P             # 8  bf16 k-tiles over the input dim (gate, bf16 mode)
CT = C // P                  # 8 column tiles per core
WSCALE = 64.0                # host pre-scale for fp8 weights

# "fp8": gate GEMM also fp8 DoubleRow (fastest; ns rel-err ~1.2e-2)
# "bf16": gate GEMM in bf16 (ns rel-err ~5e-3, +20us)
GATE_MODE = os.environ.get("BASS_GATE_MODE", "fp8")
PATCH_BAND = float(os.environ.get("BASS_PATCH_BAND", "0.04" if GATE_MODE == "fp8" else "0.02"))

F8NP = ml_dtypes.float8_e4m3
BFNP = ml_dtypes.bfloat16

_CACHE = {}


def _build(gate_mode: str):
    from contextlib import ExitStack

    from concourse import bacc, tile
    import concourse.mybir as mybir

    f32 = mybir.dt.float32
    f8 = mybir.dt.float8e4
    bf = mybir.dt.bfloat16
    AF = mybir.ActivationFunctionType
    DR = mybir.MatmulPerfMode.DoubleRow
    ALU = mybir.AluOpType

    nc = bacc.Bacc(
        "TRN2", target_bir_lowering=False, debug=False, enable_asserts=False
    )

    x8d = nc.dram_tensor("x8", [D_IN, BS], f8, kind="ExternalInput")
    s8d = nc.dram_tensor("s8", [R, BS], f8, kind="ExternalInput")
    sw9d = nc.dram_tensor("sw9", [C, BS], bf, kind="ExternalInput")
    wi8d = nc.dram_tensor("wi8", [D_IN, C], f8, kind="ExternalInput")
    wr8d = nc.dram_tensor("wr8", [R, C], f8, kind="ExternalInput")
    if gate_mode == "fp8":
        wg8d = nc.dram_tensor("wg8", [D_IN, C], f8, kind="ExternalInput")
    else:
        wgbd = nc.dram_tensor("wgb", [D_IN, C], bf, kind="ExternalInput")
        xbd = nc.dram_tensor("xb", [D_IN, BS], bf, kind="ExternalInput")
    nspd = nc.dram_tensor("nsp", [C, BS], bf, kind="ExternalOutput")

    def dr3(dram, kb, cols):
        # [256 x n] HBM block -> [128, 2, n] (partition, k-sub-row, free)
        return dram[kb * 256 : (kb + 1) * 256, cols].rearrange(
            "(s p) b -> p s b", p=P
        )

    def drb(dram, j, nkb, cols):
        # [nkb*256 x n] HBM block -> [128, nkb*2, n]: nkb k-blocks, one DMA
        return dram[j * nkb * 256 : (j + 1) * nkb * 256, cols].rearrange(
            "(k s p) b -> p (k s) b", p=P, s=2
        )

    with tile.TileContext(nc) as tc, ExitStack() as ctx:
        wp = ctx.enter_context(tc.tile_pool(name="w", bufs=1))
        mp = ctx.enter_context(tc.tile_pool(name="m", bufs=1))
        epool = ctx.enter_context(tc.tile_pool(name="e", bufs=1))
        pp = ctx.enter_context(tc.tile_pool(name="ps", bufs=8, space="PSUM"))

        # Weights/moving tiles are grouped into multi-k-block tiles so one
        # DMA covers several k-blocks (HWDGE descriptor-gen is 632ns per
        # DMA instruction and would otherwise pace the whole stream).
        # Batch sizes taper off so the last k-blocks (which gate the end of
        # the DMA-paced slice-0 state phase) arrive with minimal latency.
        WRBS = [4, 4, 4, 2, 1, 1]     # k-blocks per wr/s0 batch DMA
        WRO = [sum(WRBS[:j]) for j in range(len(WRBS))]
        S1BS = [4, 4, 4, 4]           # k-blocks per s1 batch DMA
        S1O = [sum(S1BS[:j]) for j in range(len(S1BS))]
        wi_sb = [wp.tile([P, 4, C], f8, tag=f"wi{j}", name=f"wi_sb{j}")
                 for j in range(KB_I // 2)]
        wr_sb = [wp.tile([P, 2 * n, C], f8, tag=f"wr{j}", name=f"wr_sb{j}")
                 for j, n in enumerate(WRBS)]
        if gate_mode == "fp8":
            wg_sb = wp.tile([P, 2 * KB_I, C], f8, tag="wg", name="wg_sb")
        else:
            wg_sb = [wp.tile([P, C], bf, tag=f"wg{k}", name=f"wg_sb{k}")
                     for k in range(KT_G)]

        xma = [mp.tile([P, 2 * KB_I, NB], f8, tag=f"x{sl}all",
                       name=f"xm{sl}all") for sl in range(SL)]
        xm = [[xma[sl][:, 2 * k : 2 * k + 2, :] for k in range(KB_I)]
              for sl in range(SL)]
        sm0 = [mp.tile([P, 2 * n, NB], f8, tag=f"s0_{j}", name=f"sm0_{j}")
               for j, n in enumerate(WRBS)]
        sm1 = [mp.tile([P, 2 * n, NB], f8, tag=f"s1_{j}", name=f"sm1_{j}")
               for j, n in enumerate(S1BS)]

        def s_slice(sl, k):
            # moving-tile slice covering reservoir k-block k of slice sl
            if sl == 0:
                sizes, offs, tiles = WRBS, WRO, sm0
            else:
                sizes, offs, tiles = S1BS, S1O, sm1
            for j, o in enumerate(offs):
                if o <= k < o + sizes[j]:
                    kk = k - o
                    return tiles[j][:, 2 * kk : 2 * kk + 2, :]
            raise AssertionError

        def wr_slice(k):
            for j, o in enumerate(WRO):
                if o <= k < o + WRBS[j]:
                    kk = k - o
                    return wr_sb[j][:, 2 * kk : 2 * kk + 2, :]
            raise AssertionError
        if gate_mode != "fp8":
            xbm = [[mp.tile([P, NB], bf, tag=f"xb{sl}_{k}", name=f"xbm{sl}_{k}")
                    for k in range(KT_G)] for sl in range(SL)]
        sw_sb = [mp.tile([P, CT, NB], bf, tag=f"sw{sl}", name=f"sw_sb{sl}")
                 for sl in range(SL)]

        t_sb = [[epool.tile([P, NB], bf, tag=f"t{sl}_{c}", name=f"t{sl}_{c}")
                 for c in range(CT)] for sl in range(SL)]
        g_sb = [[epool.tile([P, NB], bf, tag=f"g{sl}_{c}", name=f"g{sl}_{c}")
                 for c in range(CT)] for sl in range(SL)]
        v_sb = [[epool.tile([P, NB], bf, tag=f"v{sl}_{c}", name=f"v{sl}_{c}")
                 for c in range(CT)] for sl in range(SL)]
        ns_sb = [[epool.tile([P, 2, NB], bf, tag=f"ns{sl}_{gidx}",
                             name=f"ns{sl}_{gidx}")
                  for gidx in range(CT // 2)] for sl in range(SL)]

        def bsl(sl):
            return slice(sl * NB, (sl + 1) * NB)

        # ---- input DMA stream (SP queue), in consumption order ----
        def drbr(dram, o, nkb, cols):
            # k-blocks [o, o+nkb) of dram -> [128, nkb*2, n], one DMA
            return dram[o * 256 : (o + nkb) * 256, cols].rearrange(
                "(k s p) b -> p (k s) b", p=P, s=2
            )

        # Gate(s0) inputs lead the stream: all 8 PSUM banks are free at t=0,
        # so the gate matmuls soak up the PE while the big state weight
        # stream is still arriving.
        if gate_mode == "fp8":
            nc.sync.dma_start(
                wg_sb[:, :, 0:256], drbr(wg8d, 0, KB_I, slice(0, 256))
            )
        else:
            for k in range(KT_G):
                nc.sync.dma_start(wg_sb[k][:, 0:256],
                                  wgbd[k * P : (k + 1) * P, 0:256])
        nc.sync.dma_start(xm[0][0][:], dr3(x8d, 0, bsl(0)))
        nc.sync.dma_start(
            xma[0][:, 2:, :],
            x8d[256:D_IN, bsl(0)].rearrange("(k s p) b -> p (k s) b", p=P, s=2),
        )
        if gate_mode == "fp8":
            nc.sync.dma_start(
                wg_sb[:, :, 256:C], drbr(wg8d, 0, KB_I, slice(256, C))
            )
        else:
            for k in range(KT_G):
                nc.sync.dma_start(wg_sb[k][:, 256:C],
                                  wgbd[k * P : (k + 1) * P, 256:C])
                nc.sync.dma_start(xbm[0][k][:], xbd[k * P : (k + 1) * P, bsl(0)])
        nc.sync.dma_start(
            xma[1][:],
            x8d[:, bsl(1)].rearrange("(k s p) b -> p (k s) b", p=P, s=2),
        )
        if gate_mode != "fp8":
            for k in range(KT_G):
                nc.sync.dma_start(xbm[1][k][:], xbd[k * P : (k + 1) * P, bsl(1)])
        for j in range(KB_I // 2):
            nc.sync.dma_start(wi_sb[j][:], drbr(wi8d, 2 * j, 2, slice(0, C)))
        for j, n in enumerate(WRBS):
            nc.sync.dma_start(wr_sb[j][:], drbr(wr8d, WRO[j], n, slice(0, C)))
            nc.sync.dma_start(sm0[j][:], drbr(s8d, WRO[j], n, bsl(0)))
        for j, n in enumerate(S1BS):
            nc.sync.dma_start(sm1[j][:], drbr(s8d, S1O[j], n, bsl(1)))
        for sl in range(SL):
            nc.sync.dma_start(
                sw_sb[sl][:],
                sw9d[:, bsl(sl)].rearrange("(c p) b -> p c b", p=P),
            )

        def state_x_matmuls(st, sl, cts, kbs=None):
            for k in kbs if kbs is not None else range(KB_I):
                j, kk = divmod(k, 2)
                for c in cts:
                    nc.tensor.matmul(
                        st[c][:],
                        wi_sb[j][:, 2 * kk : 2 * kk + 2, c * P : (c + 1) * P],
                        xm[sl][k][:],
                        start=(k == 0),
                        stop=False,
                        perf_mode=DR,
                    )

        def state_s_matmuls(st, sl, cts, kbs):
            for k in kbs:
                for c in cts:
                    nc.tensor.matmul(
                        st[c][:],
                        wr_slice(k)[:, :, c * P : (c + 1) * P],
                        s_slice(sl, k),
                        start=False,
                        stop=(k == KB_R - 1),
                        perf_mode=DR,
                    )

        def gate_matmuls(gt, sl, c):
            cs = slice(c * P, (c + 1) * P)
            if gate_mode == "fp8":
                for k in range(KB_I):
                    nc.tensor.matmul(
                        gt[:], wg_sb[:, 2 * k : 2 * k + 2, cs],
                        xm[sl][k][:],
                        start=(k == 0), stop=(k == KB_I - 1), perf_mode=DR,
                    )
            else:
                for k in range(KT_G):
                    nc.tensor.matmul(
                        gt[:], wg_sb[k][:, cs], xbm[sl][k][:],
                        start=(k == 0), stop=(k == KT_G - 1),
                    )

        gate_scale = 1.0 / WSCALE if gate_mode == "fp8" else 1.0

        def emit_ns(sl, c, halves=1):
            # o = v * g into the ns staging tile, then DMA the column tile
            # out.  halves=2 pipelines the last column's epilogue in two
            # 256-wide chunks so the end-of-kernel drain chain is shorter.
            for h in range(halves):
                hs = slice(h * NB // halves, (h + 1) * NB // halves)
                nc.vector.tensor_tensor(
                    ns_sb[sl][c // 2][:, c % 2, hs], v_sb[sl][c][:, hs],
                    g_sb[sl][c][:, hs], ALU.mult,
                )
                nc.sync.dma_start(
                    nspd[c * P : (c + 1) * P, bsl(sl)][:, hs],
                    ns_sb[sl][c // 2][:, c % 2, hs],
                )

        def emit_sig(sl, c, gt, halves=1):
            for h in range(halves):
                hs = slice(h * NB // halves, (h + 1) * NB // halves)
                nc.scalar.activation(g_sb[sl][c][:, hs], gt[:, hs],
                                     AF.Sigmoid, scale=gate_scale)

        # ---- phase 0: PE warm-up.  The tensor engine clock ramps over its
        # first ~3us of continuous work; zero-matmuls during the DMA
        # lead-in get the ramp done before any real matmul issues.
        zm = mp.tile([P, 2, NB], f8, tag="zwarm", name="zm")
        nc.gpsimd.memset(zm[:], 0)
        warm_ps = pp.tile([P, NB], f32, tag="acc", name="warm_ps")
        for _ in range(16):
            nc.tensor.matmul(warm_ps[:], zm[:, :, 0:P], zm[:],
                             start=True, stop=True, perf_mode=DR)

        # ---- phase 1: BOTH slices' gates, while the state weight stream
        # arrives.  All PSUM banks are free at t=0 and the gates need only
        # wg+x (2MB); the sigmoids hand each bank onward.  This removes all
        # gate work from the back half of the kernel.
        for sl in range(SL):
            for c in range(CT):
                gt_c = pp.tile([P, NB], f32, tag="acc", name=f"gt{sl}_{c}")
                gate_matmuls(gt_c, sl, c)
                emit_sig(sl, c, gt_c)

        # ---- phase 2: slice 0 state, kb-major (DMA-streamed) ----
        st0 = [pp.tile([P, NB], f32, tag="acc", name=f"st0_{c}")
               for c in range(CT)]
        state_x_matmuls(st0, 0, range(CT))
        state_s_matmuls(st0, 0, range(CT), range(KB_R))
        for c in range(CT):
            nc.scalar.activation(t_sb[0][c][:], st0[c][:], AF.Tanh,
                                 scale=1.0 / WSCALE)
        for c in range(CT):
            nc.vector.tensor_tensor(v_sb[0][c][:], t_sb[0][c][:],
                                    sw_sb[0][:, c, :], ALU.add)
            emit_ns(0, c)

        # ---- phases 3/4: slice 1 in two ct-groups of 4.  Within a group:
        # pass A accumulates k-blocks 0..7 (needs only the first half of the
        # s1 stream), then pass B finishes k-blocks 8..15 with the gate
        # matmuls pipelined between columns.  Gate banks come from the
        # OTHER half of the PSUM pool (freed by slice-0 sigmoids / earlier
        # group), so the final gate never waits on the final tanh.
        # ---- phase 3: slice 1 state pass A (k-blocks 0..7), ct-major —
        # needs only the first half of the s1 stream.  Banks come from the
        # slice-0 tanh stream. ----
        KH = KB_R // 2
        st1 = [None] * CT
        for c in range(CT):
            st1[c] = pp.tile([P, NB], f32, tag="acc", name=f"st1_{c}")
            state_x_matmuls(st1, 1, [c])
            state_s_matmuls(st1, 1, [c], range(KH))

        # ---- phase 4: pass B (k-blocks 8..15) + epilogue per column.
        # Gates already done, so the tail is only the final tanh+v+ns. ----
        for c in range(CT):
            last = (c == CT - 1)
            state_s_matmuls(st1, 1, [c], range(KH, KB_R))
            if not last:
                nc.scalar.activation(t_sb[1][c][:], st1[c][:], AF.Tanh,
                                     scale=1.0 / WSCALE)
                nc.vector.tensor_tensor(v_sb[1][c][:], t_sb[1][c][:],
                                        sw_sb[1][:, c, :], ALU.add)
                emit_ns(1, c)
            else:
                # final column: per-half tanh+v+ns+DMA chains so the drain
                # pipeline overlaps the second half's compute
                for h in range(2):
                    hs = slice(h * NB // 2, (h + 1) * NB // 2)
                    nc.scalar.activation(t_sb[1][c][:, hs], st1[c][:, hs],
                                         AF.Tanh, scale=1.0 / WSCALE)
                    nc.vector.tensor_tensor(v_sb[1][c][:, hs],
                                            t_sb[1][c][:, hs],
                                            sw_sb[1][:, c, hs], ALU.add)
                    nc.vector.tensor_tensor(
                        ns_sb[1][c // 2][:, c % 2, hs],
                        v_sb[1][c][:, hs], g_sb[1][c][:, hs], ALU.mult,
                    )
                    nc.sync.dma_start(
                        nspd[c * P : (c + 1) * P, bsl(1)][:, hs],
                        ns_sb[1][c // 2][:, c % 2, hs],
                    )

    nc.compile()
    return nc


def _get_program():
    if GATE_MODE not in _CACHE:
        _CACHE[GATE_MODE] = _build(GATE_MODE)
    return _CACHE[GATE_MODE]


def kernel(inputs, prev_output, reservoir_state, input_weights,
           reservoir_weights, gate_weights):
    from concourse.bass_utils import run_bass_kernel_spmd

    nc = _get_program()

    x = np.ascontiguousarray(np.asarray(inputs, dtype=np.float32))
    s = np.ascontiguousarray(np.asarray(reservoir_state, dtype=np.float32))
    w_in = np.asarray(input_weights, dtype=np.float32)
    w_res = np.asarray(reservoir_weights, dtype=np.float32)
    w_gate = np.asarray(gate_weights, dtype=np.float32)

    xT = np.ascontiguousarray(x.T)           # [D_IN, B]
    sT = np.ascontiguousarray(s.T)           # [R, B]
    x8 = xT.astype(F8NP)
    s8 = sT.astype(F8NP)
    sw9 = (9.0 * sT).astype(BFNP)            # [R, B]
    wi8 = (w_in * WSCALE).astype(F8NP)       # [D_IN, R]
    wr8 = (w_res * WSCALE).astype(F8NP)      # [R, R]
    if GATE_MODE == "fp8":
        wg8 = (w_gate * WSCALE).astype(F8NP)
    else:
        wgb = w_gate.astype(BFNP)
        xb = xT.astype(BFNP)

    in_maps = []
    for core in range(N_CORES):
        bg, cg = divmod(core, CGRP)
        cs = slice(cg * C, (cg + 1) * C)
        bs_ = slice(bg * BS, (bg + 1) * BS)
        m = {
            "x8": np.ascontiguousarray(x8[:, bs_]),
            "s8": np.ascontiguousarray(s8[:, bs_]),
            "sw9": np.ascontiguousarray(sw9[cs, bs_]),
            "wi8": np.ascontiguousarray(wi8[:, cs]),
            "wr8": np.ascontiguousarray(wr8[:, cs]),
        }
        if GATE_MODE == "fp8":
            m["wg8"] = np.ascontiguousarray(wg8[:, cs])
        else:
            m["wgb"] = np.ascontiguousarray(wgb[:, cs])
            m["xb"] = np.ascontiguousarray(xb[:, bs_])
        in_maps.append(m)

    res = run_bass_kernel_spmd(nc, in_maps, list(range(N_CORES)))

    nsp = np.empty((R, B), dtype=np.float32)  # 10 * new_state, transposed
    for core in range(N_CORES):
        bg, cg = divmod(core, CGRP)
        nsp[cg * C : (cg + 1) * C, bg * BS : (bg + 1) * BS] = (
            res.results[core]["nsp"].astype(np.float32)
        )
    new_state = np.ascontiguousarray(nsp.T) * np.float32(0.1)  # [B, R]

    # Re-evaluate borderline elements (|ns - 0.5| < band) in full precision
    # so fp8/bf16 quantization error cannot flip spikes at the threshold.
    bi, rj = np.nonzero(np.abs(new_state - 0.5) < PATCH_BAND)
    if bi.size:
        CH = 32768
        for lo in range(0, bi.size, CH):
            bc, rc = bi[lo : lo + CH], rj[lo : lo + CH]
            xb_ = x[bc]                       # [n, D_IN]
            sb_ = s[bc]                       # [n, R]
            acc = np.einsum("ij,ji->i", xb_, w_in[:, rc], optimize=True)
            acc += np.einsum("ij,ji->i", sb_, w_res[:, rc], optimize=True)
            z = np.einsum("ij,ji->i", xb_, w_gate[:, rc], optimize=True)
            gate = 1.0 / (1.0 + np.exp(-z.astype(np.float64)))
            ns_fix = (0.9 * sb_[np.arange(bc.size), rc].astype(np.float64)
                      + 0.1 * np.tanh(acc.astype(np.float64))) * gate
            new_state[bc, rc] = ns_fix.astype(np.float32)

    output = (new_state > np.float32(0.5)).astype(np.float32)
    return output, new_state


# revision 53
# speedup vs baseline: 3.6737x; 1.0244x over previous
"""Gated spiking reservoir step — Trainium2 Bass kernel (8 NeuronCores).

Math (per reference):
    ic   = inputs @ input_weights                  # [B, R]
    rc   = reservoir_state @ reservoir_weights     # [B, R]
    gate = sigmoid(inputs @ gate_weights)          # [B, R]
    ns   = (0.9 * reservoir_state + 0.1 * tanh(ic + rc)) * gate
    out  = (ns > 0.5) ? 1.0 : 0.0
    returns (out, ns)

Strategy
--------
2-D sharding: 4 column groups x 2 batch groups.  Each core owns a
[1024-column x 1024-batch] block of the output and loads the matching
weight column slice plus the activation batch slice (transposed to [K, B]
so the contraction dim lands on SBUF partitions).

All three GEMMs run in fp8 (e4m3) with MatmulPerfMode.DoubleRow: each
matmul instruction contracts K=256 (two 128-row sub-tiles) at 0.5
cycles/row -- 4x the fp32r/bf16 rate.  Weights are pre-scaled by 64 on
host so their 0.02-std distribution lands in e4m3's normal range; the
64x is divided back out by the activation-function input scale.

Epilogue (per [128 x 512] output tile):
    t  = bf16(tanh(state_psum / 64))       ScalarE, PSUM -> SBUF
    g  = bf16(sigmoid(gate_psum / 64))     ScalarE
    v  = t + sw9                           VectorE tensor_tensor (bf16, 2x)
    o  = v * g                             VectorE tensor_tensor (bf16, 2x)
where sw9 = bf16(9 * s) is host-prepared, so the device returns
o = (9s + tanh(.)) * sigmoid(.) = 10 * new_state.  The host multiplies
by 0.1, thresholds at 0.5 for the spike output, and re-evaluates the
few borderline elements (|ns - 0.5| < band) in full precision to absorb
the fp8 quantization error at the spike threshold.
"""

import os
import sys

if "/opt/trn_rl_repo" not in sys.path:
    sys.path.insert(0, "/opt/trn_rl_repo")

import numpy as np
import ml_dtypes

B, D_IN, R = 2048, 1024, 4096
N_CORES = 8
CGRP, BGRP = 4, 2            # column groups x batch groups
C = R // CGRP                # 1024 output columns per core
BS = B // BGRP               # 1024 batch rows per core
P = 128                      # SBUF/PSUM partitions
NB = 512                     # moving free dim per matmul / PSUM bank
SL = BS // NB                # 2 batch slices per core
KB_I = D_IN // 256           # 4  fp8 DoubleRow k-blocks over the input dim
KB_R = R // 256              # 16 fp8 DoubleRow k-blocks over the reservoir dim
KT_G = D_IN // P             # 8  bf16 k-tiles over the input dim (gate, bf16 mode)
CT = C // P                  # 8 column tiles per core
WSCALE = 64.0                # host pre-scale for fp8 weights

# "fp8": gate GEMM also fp8 DoubleRow (fastest; ns rel-err ~1.2e-2)
# "bf16": gate GEMM in bf16 (ns rel-err ~5e-3, +20us)
GATE_MODE = os.environ.get("BASS_GATE_MODE", "fp8")
PATCH_BAND = float(os.environ.get("BASS_PATCH_BAND", "0.04" if GATE_MODE == "fp8" else "0.02"))

F8NP = ml_dtypes.float8_e4m3
BFNP = ml_dtypes.bfloat16

_CACHE = {}


def _build(gate_mode: str):
    from contextlib import ExitStack

    from concourse import bacc, tile
    import concourse.mybir as mybir

    f32 = mybir.dt.float32
    f8 = mybir.dt.float8e4
    bf = mybir.dt.bfloat16
    AF = mybir.ActivationFunctionType
    DR = mybir.MatmulPerfMode.DoubleRow
    ALU = mybir.AluOpType

    nc = bacc.Bacc(
        "TRN2", target_bir_lowering=False, debug=False, enable_asserts=False
    )

    x8d = nc.dram_tensor("x8", [D_IN, BS], f8, kind="ExternalInput")
    s8d = nc.dram_tensor("s8", [R, BS], f8, kind="ExternalInput")
    sw9d = nc.dram_tensor("sw9", [C, BS], bf, kind="ExternalInput")
    wi8d = nc.dram_tensor("wi8", [D_IN, C], f8, kind="ExternalInput")
    wr8d = nc.dram_tensor("wr8", [R, C], f8, kind="ExternalInput")
    if gate_mode == "fp8":
        wg8d = nc.dram_tensor("wg8", [D_IN, C], f8, kind="ExternalInput")
    else:
        wgbd = nc.dram_tensor("wgb", [D_IN, C], bf, kind="ExternalInput")
        xbd = nc.dram_tensor("xb", [D_IN, BS], bf, kind="ExternalInput")
    nspd = nc.dram_tensor("nsp", [C, BS], bf, kind="ExternalOutput")

    def dr3(dram, kb, cols):
        # [256 x n] HBM block -> [128, 2, n] (partition, k-sub-row, free)
        return dram[kb * 256 : (kb + 1) * 256, cols].rearrange(
            "(s p) b -> p s b", p=P
        )

    def drb(dram, j, nkb, cols):
        # [nkb*256 x n] HBM block -> [128, nkb*2, n]: nkb k-blocks, one DMA
        return dram[j * nkb * 256 : (j + 1) * nkb * 256, cols].rearrange(
            "(k s p) b -> p (k s) b", p=P, s=2
        )

    with tile.TileContext(nc) as tc, ExitStack() as ctx:
        wp = ctx.enter_context(tc.tile_pool(name="w", bufs=1))
        mp = ctx.enter_context(tc.tile_pool(name="m", bufs=1))
        epool = ctx.enter_context(tc.tile_pool(name="e", bufs=1))
        pp = ctx.enter_context(tc.tile_pool(name="ps", bufs=8, space="PSUM"))

        # Weights/moving tiles are grouped into multi-k-block tiles so one
        # DMA covers several k-blocks (HWDGE descriptor-gen is 632ns per
        # DMA instruction and would otherwise pace the whole stream).
        # Batch sizes taper off so the last k-blocks (which gate the end of
        # the DMA-paced slice-0 state phase) arrive with minimal latency.
        WRBS = [4, 4, 4, 2, 1, 1]     # k-blocks per wr/s0 batch DMA
        WRO = [sum(WRBS[:j]) for j in range(len(WRBS))]
        S1BS = [4, 4, 4, 4]           # k-blocks per s1 batch DMA
        S1O = [sum(S1BS[:j]) for j in range(len(S1BS))]
        wi_sb = [wp.tile([P, 4, C], f8, tag=f"wi{j}", name=f"wi_sb{j}")
                 for j in range(KB_I // 2)]
        wr_sb = [wp.tile([P, 2 * n, C], f8, tag=f"wr{j}", name=f"wr_sb{j}")
                 for j, n in enumerate(WRBS)]
        if gate_mode == "fp8":
            wg_sb = wp.tile([P, 2 * KB_I, C], f8, tag="wg", name="wg_sb")
        else:
            wg_sb = [wp.tile([P, C], bf, tag=f"wg{k}", name=f"wg_sb{k}")
                     for k in range(KT_G)]

        xma = [mp.tile([P, 2 * KB_I, NB], f8, tag=f"x{sl}all",
                       name=f"xm{sl}all") for sl in range(SL)]
        xm = [[xma[sl][:, 2 * k : 2 * k + 2, :] for k in range(KB_I)]
              for sl in range(SL)]
        sm0 = [mp.tile([P, 2 * n, NB], f8, tag=f"s0_{j}", name=f"sm0_{j}")
               for j, n in enumerate(WRBS)]
        sm1 = [mp.tile([P, 2 * n, NB], f8, tag=f"s1_{j}", name=f"sm1_{j}")
               for j, n in enumerate(S1BS)]

        def s_slice(sl, k):
            # moving-tile slice covering reservoir k-block k of slice sl
            if sl == 0:
                sizes, offs, tiles = WRBS, WRO, sm0
            else:
                sizes, offs, tiles = S1BS, S1O, sm1
            for j, o in enumerate(offs):
                if o <= k < o + sizes[j]:
                    kk = k - o
                    return tiles[j][:, 2 * kk : 2 * kk + 2, :]
            raise AssertionError

        def wr_slice(k):
            for j, o in enumerate(WRO):
                if o <= k < o + WRBS[j]:
                    kk = k - o
                    return wr_sb[j][:, 2 * kk : 2 * kk + 2, :]
            raise AssertionError
        if gate_mode != "fp8":
            xbm = [[mp.tile([P, NB], bf, tag=f"xb{sl}_{k}", name=f"xbm{sl}_{k}")
                    for k in range(KT_G)] for sl in range(SL)]
        sw_sb = [mp.tile([P, CT, NB], bf, tag=f"sw{sl}", name=f"sw_sb{sl}")
                 for sl in range(SL)]

        t_sb = [[epool.tile([P, NB], bf, tag=f"t{sl}_{c}", name=f"t{sl}_{c}")
                 for c in range(CT)] for sl in range(SL)]
        g_sb = [[epool.tile([P, NB], bf, tag=f"g{sl}_{c}", name=f"g{sl}_{c}")
                 for c in range(CT)] for sl in range(SL)]
        v_sb = [[epool.tile([P, NB], bf, tag=f"v{sl}_{c}", name=f"v{sl}_{c}")
                 for c in range(CT)] for sl in range(SL)]
        ns_sb = [[epool.tile([P, 2, NB], bf, tag=f"ns{sl}_{gidx}",
                             name=f"ns{sl}_{gidx}")
                  for gidx in range(CT // 2)] for sl in range(SL)]

        def bsl(sl):
            return slice(sl * NB, (sl + 1) * NB)

        # ---- input DMA stream (SP queue), in consumption order ----
        def drbr(dram, o, nkb, cols):
            # k-blocks [o, o+nkb) of dram -> [128, nkb*2, n], one DMA
            return dram[o * 256 : (o + nkb) * 256, cols].rearrange(
                "(k s p) b -> p (k s) b", p=P, s=2
            )

        # Gate(s0) inputs lead the stream: all 8 PSUM banks are free at t=0,
        # so the gate matmuls soak up the PE while the big state weight
        # stream is still arriving.
        if gate_mode == "fp8":
            nc.sync.dma_start(
                wg_sb[:, :, 0:512], drbr(wg8d, 0, KB_I, slice(0, 512))
            )
        else:
            for k in range(KT_G):
                nc.sync.dma_start(wg_sb[k][:, 0:256],
                                  wgbd[k * P : (k + 1) * P, 0:256])
        nc.sync.dma_start(xm[0][0][:], dr3(x8d, 0, bsl(0)))
        nc.sync.dma_start(
            xma[0][:, 2:, :],
            x8d[256:D_IN, bsl(0)].rearrange("(k s p) b -> p (k s) b", p=P, s=2),
        )
        if gate_mode == "fp8":
            nc.sync.dma_start(
                wg_sb[:, :, 512:C], drbr(wg8d, 0, KB_I, slice(512, C))
            )
        else:
            for k in range(KT_G):
                nc.sync.dma_start(wg_sb[k][:, 256:C],
                                  wgbd[k * P : (k + 1) * P, 256:C])
                nc.sync.dma_start(xbm[0][k][:], xbd[k * P : (k + 1) * P, bsl(0)])
        nc.sync.dma_start(
            xma[1][:],
            x8d[:, bsl(1)].rearrange("(k s p) b -> p (k s) b", p=P, s=2),
        )
        if gate_mode != "fp8":
            for k in range(KT_G):
                nc.sync.dma_start(xbm[1][k][:], xbd[k * P : (k + 1) * P, bsl(1)])
        for j in range(KB_I // 2):
            nc.sync.dma_start(wi_sb[j][:], drbr(wi8d, 2 * j, 2, slice(0, C)))
        for j, n in enumerate(WRBS):
            nc.sync.dma_start(wr_sb[j][:], drbr(wr8d, WRO[j], n, slice(0, C)))
            nc.sync.dma_start(sm0[j][:], drbr(s8d, WRO[j], n, bsl(0)))
        for j, n in enumerate(S1BS):
            nc.sync.dma_start(sm1[j][:], drbr(s8d, S1O[j], n, bsl(1)))
        for sl in range(SL):
            nc.sync.dma_start(
                sw_sb[sl][:],
                sw9d[:, bsl(sl)].rearrange("(c p) b -> p c b", p=P),
            )

        def state_x_matmuls(st, sl, cts, kbs=None):
            for k in kbs if kbs is not None else range(KB_I):
                j, kk = divmod(k, 2)
                for c in cts:
                    nc.tensor.matmul(
                        st[c][:],
                        wi_sb[j][:, 2 * kk : 2 * kk + 2, c * P : (c + 1) * P],
                        xm[sl][k][:],
                        start=(k == 0),
                        stop=False,
                        perf_mode=DR,
                    )

        def state_s_matmuls(st, sl, cts, kbs):
            for k in kbs:
                for c in cts:
                    nc.tensor.matmul(
                        st[c][:],
                        wr_slice(k)[:, :, c * P : (c + 1) * P],
                        s_slice(sl, k),
                        start=False,
                        stop=(k == KB_R - 1),
                        perf_mode=DR,
                    )

        def gate_matmuls(gt, sl, c):
            cs = slice(c * P, (c + 1) * P)
            if gate_mode == "fp8":
                for k in range(KB_I):
                    nc.tensor.matmul(
                        gt[:], wg_sb[:, 2 * k : 2 * k + 2, cs],
                        xm[sl][k][:],
                        start=(k == 0), stop=(k == KB_I - 1), perf_mode=DR,
                    )
            else:
                for k in range(KT_G):
                    nc.tensor.matmul(
                        gt[:], wg_sb[k][:, cs], xbm[sl][k][:],
                        start=(k == 0), stop=(k == KT_G - 1),
                    )

        gate_scale = 1.0 / WSCALE if gate_mode == "fp8" else 1.0

        def emit_ns(sl, c, halves=1):
            # o = v * g into the ns staging tile, then DMA the column tile
            # out.  halves=2 pipelines the last column's epilogue in two
            # 256-wide chunks so the end-of-kernel drain chain is shorter.
            for h in range(halves):
                hs = slice(h * NB // halves, (h + 1) * NB // halves)
                nc.vector.tensor_tensor(
                    ns_sb[sl][c // 2][:, c % 2, hs], v_sb[sl][c][:, hs],
                    g_sb[sl][c][:, hs], ALU.mult,
                )
                nc.sync.dma_start(
                    nspd[c * P : (c + 1) * P, bsl(sl)][:, hs],
                    ns_sb[sl][c // 2][:, c % 2, hs],
                )

        def emit_sig(sl, c, gt, halves=1):
            for h in range(halves):
                hs = slice(h * NB // halves, (h + 1) * NB // halves)
                nc.scalar.activation(g_sb[sl][c][:, hs], gt[:, hs],
                                     AF.Sigmoid, scale=gate_scale)

        # ---- phase 0: PE warm-up.  The tensor engine clock ramps over its
        # first ~3us of continuous work; zero-matmuls during the DMA
        # lead-in get the ramp done before any real matmul issues.
        zm = mp.tile([P, 2, NB], f8, tag="zwarm", name="zm")
        nc.gpsimd.memset(zm[:], 0)
        warm_ps = pp.tile([P, NB], f32, tag="acc", name="warm_ps")
        for _ in range(16):
            nc.tensor.matmul(warm_ps[:], zm[:, :, 0:P], zm[:],
                             start=True, stop=True, perf_mode=DR)

        # ---- phase 1: BOTH slices' gates, while the state weight stream
        # arrives.  All PSUM banks are free at t=0 and the gates need only
        # wg+x (2MB); the sigmoids hand each bank onward.  This removes all
        # gate work from the back half of the kernel.
        for sl in range(SL):
            for c in range(CT):
                gt_c = pp.tile([P, NB], f32, tag="acc", name=f"gt{sl}_{c}")
                gate_matmuls(gt_c, sl, c)
                emit_sig(sl, c, gt_c)

        # ---- phase 2: slice 0 state, kb-major (DMA-streamed) ----
        st0 = [pp.tile([P, NB], f32, tag="acc", name=f"st0_{c}")
               for c in range(CT)]
        state_x_matmuls(st0, 0, range(CT))
        state_s_matmuls(st0, 0, range(CT), range(KB_R))
        for c in range(CT):
            nc.scalar.activation(t_sb[0][c][:], st0[c][:], AF.Tanh,
                                 scale=1.0 / WSCALE)
        for c in range(CT):
            nc.vector.tensor_tensor(v_sb[0][c][:], t_sb[0][c][:],
                                    sw_sb[0][:, c, :], ALU.add)
            emit_ns(0, c)

        # ---- phases 3/4: slice 1 in two ct-groups of 4.  Within a group:
        # pass A accumulates k-blocks 0..7 (needs only the first half of the
        # s1 stream), then pass B finishes k-blocks 8..15 with the gate
        # matmuls pipelined between columns.  Gate banks come from the
        # OTHER half of the PSUM pool (freed by slice-0 sigmoids / earlier
        # group), so the final gate never waits on the final tanh.
        # ---- phase 3: slice 1 state pass A (k-blocks 0..7), ct-major —
        # needs only the first half of the s1 stream.  Banks come from the
        # slice-0 tanh stream. ----
        KH = KB_R // 2
        st1 = [None] * CT
        for c in range(CT):
            st1[c] = pp.tile([P, NB], f32, tag="acc", name=f"st1_{c}")
            state_x_matmuls(st1, 1, [c])
            state_s_matmuls(st1, 1, [c], range(KH))

        # ---- phase 4: pass B (k-blocks 8..15) + epilogue per column.
        # Gates already done, so the tail is only the final tanh+v+ns. ----
        for c in range(CT):
            last = (c == CT - 1)
            state_s_matmuls(st1, 1, [c], range(KH, KB_R))
            if not last:
                nc.scalar.activation(t_sb[1][c][:], st1[c][:], AF.Tanh,
                                     scale=1.0 / WSCALE)
                nc.vector.tensor_tensor(v_sb[1][c][:], t_sb[1][c][:],
                                        sw_sb[1][:, c, :], ALU.add)
                emit_ns(1, c)
            else:
                # final column: per-half tanh+v+ns+DMA chains so the drain
                # pipeline overlaps the second half's compute
                nc.scalar.activation(t_sb[1][c][:], st1[c][:], AF.Tanh,
                                     scale=1.0 / WSCALE)
                for h in range(2):
                    hs = slice(h * NB // 2, (h + 1) * NB // 2)
                    nc.vector.tensor_tensor(v_sb[1][c][:, hs],
                                            t_sb[1][c][:, hs],
                                            sw_sb[1][:, c, hs], ALU.add)
                    nc.vector.tensor_tensor(
                        ns_sb[1][c // 2][:, c % 2, hs],
                        v_sb[1][c][:, hs], g_sb[1][c][:, hs], ALU.mult,
                    )
                    nc.sync.dma_start(
                        nspd[c * P : (c + 1) * P, bsl(1)][:, hs],
                        ns_sb[1][c // 2][:, c % 2, hs],
                    )

    nc.compile()
    return nc


def _get_program():
    if GATE_MODE not in _CACHE:
        _CACHE[GATE_MODE] = _build(GATE_MODE)
    return _CACHE[GATE_MODE]


def kernel(inputs, prev_output, reservoir_state, input_weights,
           reservoir_weights, gate_weights):
    from concourse.bass_utils import run_bass_kernel_spmd

    nc = _get_program()

    x = np.ascontiguousarray(np.asarray(inputs, dtype=np.float32))
    s = np.ascontiguousarray(np.asarray(reservoir_state, dtype=np.float32))
    w_in = np.asarray(input_weights, dtype=np.float32)
    w_res = np.asarray(reservoir_weights, dtype=np.float32)
    w_gate = np.asarray(gate_weights, dtype=np.float32)

    xT = np.ascontiguousarray(x.T)           # [D_IN, B]
    sT = np.ascontiguousarray(s.T)           # [R, B]
    x8 = xT.astype(F8NP)
    s8 = sT.astype(F8NP)
    sw9 = (9.0 * sT).astype(BFNP)            # [R, B]
    wi8 = (w_in * WSCALE).astype(F8NP)       # [D_IN, R]
    wr8 = (w_res * WSCALE).astype(F8NP)      # [R, R]
    if GATE_MODE == "fp8":
        wg8 = (w_gate * WSCALE).astype(F8NP)
    else:
        wgb = w_gate.astype(BFNP)
        xb = xT.astype(BFNP)

    in_maps = []
    for core in range(N_CORES):
        bg, cg = divmod(core, CGRP)
        cs = slice(cg * C, (cg + 1) * C)
        bs_ = slice(bg * BS, (bg + 1) * BS)
        m = {
            "x8": np.ascontiguousarray(x8[:, bs_]),
            "s8": np.ascontiguousarray(s8[:, bs_]),
            "sw9": np.ascontiguousarray(sw9[cs, bs_]),
            "wi8": np.ascontiguousarray(wi8[:, cs]),
            "wr8": np.ascontiguousarray(wr8[:, cs]),
        }
        if GATE_MODE == "fp8":
            m["wg8"] = np.ascontiguousarray(wg8[:, cs])
        else:
            m["wgb"] = np.ascontiguousarray(wgb[:, cs])
            m["xb"] = np.ascontiguousarray(xb[:, bs_])
        in_maps.append(m)

    res = run_bass_kernel_spmd(nc, in_maps, list(range(N_CORES)))

    nsp = np.empty((R, B), dtype=np.float32)  # 10 * new_state, transposed
    for core in range(N_CORES):
        bg, cg = divmod(core, CGRP)
        nsp[cg * C : (cg + 1) * C, bg * BS : (bg + 1) * BS] = (
            res.results[core]["nsp"].astype(np.float32)
        )
    new_state = np.ascontiguousarray(nsp.T) * np.float32(0.1)  # [B, R]

    # Re-evaluate borderline elements (|ns - 0.5| < band) in full precision
    # so fp8/bf16 quantization error cannot flip spikes at the threshold.
    bi, rj = np.nonzero(np.abs(new_state - 0.5) < PATCH_BAND)
    if bi.size:
        CH = 32768
        for lo in range(0, bi.size, CH):
            bc, rc = bi[lo : lo + CH], rj[lo : lo + CH]
            xb_ = x[bc]                       # [n, D_IN]
            sb_ = s[bc]                       # [n, R]
            acc = np.einsum("ij,ji->i", xb_, w_in[:, rc], optimize=True)
            acc += np.einsum("ij,ji->i", sb_, w_res[:, rc], optimize=True)
            z = np.einsum("ij,ji->i", xb_, w_gate[:, rc], optimize=True)
            gate = 1.0 / (1.0 + np.exp(-z.astype(np.float64)))
            ns_fix = (0.9 * sb_[np.arange(bc.size), rc].astype(np.float64)
                      + 0.1 * np.tanh(acc.astype(np.float64))) * gate
            new_state[bc, rc] = ns_fix.astype(np.float32)

    output = (new_state > np.float32(0.5)).astype(np.float32)
    return output, new_state


# revision 57
# speedup vs baseline: 3.6951x; 1.0058x over previous
"""Gated spiking reservoir step — Trainium2 Bass kernel (8 NeuronCores).

Math (per reference):
    ic   = inputs @ input_weights                  # [B, R]
    rc   = reservoir_state @ reservoir_weights     # [B, R]
    gate = sigmoid(inputs @ gate_weights)          # [B, R]
    ns   = (0.9 * reservoir_state + 0.1 * tanh(ic + rc)) * gate
    out  = (ns > 0.5) ? 1.0 : 0.0
    returns (out, ns)

Strategy
--------
2-D sharding: 4 column groups x 2 batch groups.  Each core owns a
[1024-column x 1024-batch] block of the output and loads the matching
weight column slice plus the activation batch slice (transposed to [K, B]
so the contraction dim lands on SBUF partitions).

All three GEMMs run in fp8 (e4m3) with MatmulPerfMode.DoubleRow: each
matmul instruction contracts K=256 (two 128-row sub-tiles) at 0.5
cycles/row -- 4x the fp32r/bf16 rate.  Weights are pre-scaled by 64 on
host so their 0.02-std distribution lands in e4m3's normal range; the
64x is divided back out by the activation-function input scale.

Epilogue (per [128 x 512] output tile):
    t  = bf16(tanh(state_psum / 64))       ScalarE, PSUM -> SBUF
    g  = bf16(sigmoid(gate_psum / 64))     ScalarE
    v  = t + sw9                           VectorE tensor_tensor (bf16, 2x)
    o  = v * g                             VectorE tensor_tensor (bf16, 2x)
where sw9 = bf16(9 * s) is host-prepared, so the device returns
o = (9s + tanh(.)) * sigmoid(.) = 10 * new_state.  The host multiplies
by 0.1, thresholds at 0.5 for the spike output, and re-evaluates the
few borderline elements (|ns - 0.5| < band) in full precision to absorb
the fp8 quantization error at the spike threshold.
"""

import os
import sys

if "/opt/trn_rl_repo" not in sys.path:
    sys.path.insert(0, "/opt/trn_rl_repo")

import numpy as np
import ml_dtypes

B, D_IN, R = 2048, 1024, 4096
N_CORES = 8
CGRP, BGRP = 4, 2            # column groups x batch groups
C = R // CGRP                # 1024 output columns per core
BS = B // BGRP               # 1024 batch rows per core
P = 128                      # SBUF/PSUM partitions
NB = 512                     # moving free dim per matmul / PSUM bank
SL = BS // NB                # 2 batch slices per core
KB_I = D_IN // 256           # 4  fp8 DoubleRow k-blocks over the input dim
KB_R = R // 256              # 16 fp8 DoubleRow k-blocks over the reservoir dim
KT_G = D_IN // P             # 8  bf16 k-tiles over the input dim (gate, bf16 mode)
CT = C // P                  # 8 column tiles per core
WSCALE = 64.0                # host pre-scale for fp8 weights

# "fp8": gate GEMM also fp8 DoubleRow (fastest; ns rel-err ~1.2e-2)
# "bf16": gate GEMM in bf16 (ns rel-err ~5e-3, +20us)
GATE_MODE = os.environ.get("BASS_GATE_MODE", "fp8")
PATCH_BAND = float(os.environ.get("BASS_PATCH_BAND", "0.04" if GATE_MODE == "fp8" else "0.02"))

F8NP = ml_dtypes.float8_e4m3
BFNP = ml_dtypes.bfloat16

_CACHE = {}


def _build(gate_mode: str):
    from contextlib import ExitStack

    from concourse import bacc, tile
    import concourse.mybir as mybir

    f32 = mybir.dt.float32
    f8 = mybir.dt.float8e4
    bf = mybir.dt.bfloat16
    AF = mybir.ActivationFunctionType
    DR = mybir.MatmulPerfMode.DoubleRow
    ALU = mybir.AluOpType

    nc = bacc.Bacc(
        "TRN2", target_bir_lowering=False, debug=False, enable_asserts=False
    )

    x8d = nc.dram_tensor("x8", [D_IN, BS], f8, kind="ExternalInput")
    s8d = nc.dram_tensor("s8", [R, BS], f8, kind="ExternalInput")
    sw9d = nc.dram_tensor("sw9", [C, BS], bf, kind="ExternalInput")
    wi8d = nc.dram_tensor("wi8", [D_IN, C], f8, kind="ExternalInput")
    wr8d = nc.dram_tensor("wr8", [R, C], f8, kind="ExternalInput")
    if gate_mode == "fp8":
        wg8d = nc.dram_tensor("wg8", [D_IN, C], f8, kind="ExternalInput")
    else:
        wgbd = nc.dram_tensor("wgb", [D_IN, C], bf, kind="ExternalInput")
        xbd = nc.dram_tensor("xb", [D_IN, BS], bf, kind="ExternalInput")
    nspd = nc.dram_tensor("nsp", [C, BS], bf, kind="ExternalOutput")

    def dr3(dram, kb, cols):
        # [256 x n] HBM block -> [128, 2, n] (partition, k-sub-row, free)
        return dram[kb * 256 : (kb + 1) * 256, cols].rearrange(
            "(s p) b -> p s b", p=P
        )

    def drb(dram, j, nkb, cols):
        # [nkb*256 x n] HBM block -> [128, nkb*2, n]: nkb k-blocks, one DMA
        return dram[j * nkb * 256 : (j + 1) * nkb * 256, cols].rearrange(
            "(k s p) b -> p (k s) b", p=P, s=2
        )

    with tile.TileContext(nc) as tc, ExitStack() as ctx:
        wp = ctx.enter_context(tc.tile_pool(name="w", bufs=1))
        mp = ctx.enter_context(tc.tile_pool(name="m", bufs=1))
        epool = ctx.enter_context(tc.tile_pool(name="e", bufs=1))
        pp = ctx.enter_context(tc.tile_pool(name="ps", bufs=8, space="PSUM"))

        # Weights/moving tiles are grouped into multi-k-block tiles so one
        # DMA covers several k-blocks (HWDGE descriptor-gen is 632ns per
        # DMA instruction and would otherwise pace the whole stream).
        # Batch sizes taper off so the last k-blocks (which gate the end of
        # the DMA-paced slice-0 state phase) arrive with minimal latency.
        WRBS = [4, 4, 4, 2, 1, 1]     # k-blocks per wr/s0 batch DMA
        WRO = [sum(WRBS[:j]) for j in range(len(WRBS))]
        S1BS = [4, 4, 4, 4]           # k-blocks per s1 batch DMA
        S1O = [sum(S1BS[:j]) for j in range(len(S1BS))]
        wi_sb = [wp.tile([P, 4, C], f8, tag=f"wi{j}", name=f"wi_sb{j}")
                 for j in range(KB_I // 2)]
        wr_sb = [wp.tile([P, 2 * n, C], f8, tag=f"wr{j}", name=f"wr_sb{j}")
                 for j, n in enumerate(WRBS)]
        if gate_mode == "fp8":
            wg_sb = wp.tile([P, 2 * KB_I, C], f8, tag="wg", name="wg_sb")
        else:
            wg_sb = [wp.tile([P, C], bf, tag=f"wg{k}", name=f"wg_sb{k}")
                     for k in range(KT_G)]

        xma = [mp.tile([P, 2 * KB_I, NB], f8, tag=f"x{sl}all",
                       name=f"xm{sl}all") for sl in range(SL)]
        xm = [[xma[sl][:, 2 * k : 2 * k + 2, :] for k in range(KB_I)]
              for sl in range(SL)]
        sm0 = [mp.tile([P, 2 * n, NB], f8, tag=f"s0_{j}", name=f"sm0_{j}")
               for j, n in enumerate(WRBS)]
        sm1 = [mp.tile([P, 2 * n, NB], f8, tag=f"s1_{j}", name=f"sm1_{j}")
               for j, n in enumerate(S1BS)]

        def s_slice(sl, k):
            # moving-tile slice covering reservoir k-block k of slice sl
            if sl == 0:
                sizes, offs, tiles = WRBS, WRO, sm0
            else:
                sizes, offs, tiles = S1BS, S1O, sm1
            for j, o in enumerate(offs):
                if o <= k < o + sizes[j]:
                    kk = k - o
                    return tiles[j][:, 2 * kk : 2 * kk + 2, :]
            raise AssertionError

        def wr_slice(k):
            for j, o in enumerate(WRO):
                if o <= k < o + WRBS[j]:
                    kk = k - o
                    return wr_sb[j][:, 2 * kk : 2 * kk + 2, :]
            raise AssertionError
        if gate_mode != "fp8":
            xbm = [[mp.tile([P, NB], bf, tag=f"xb{sl}_{k}", name=f"xbm{sl}_{k}")
                    for k in range(KT_G)] for sl in range(SL)]
        sw_sb = [mp.tile([P, CT, NB], bf, tag=f"sw{sl}", name=f"sw_sb{sl}")
                 for sl in range(SL)]

        t_sb = [[epool.tile([P, NB], bf, tag=f"t{sl}_{c}", name=f"t{sl}_{c}")
                 for c in range(CT)] for sl in range(SL)]
        g_sb = [[epool.tile([P, NB], bf, tag=f"g{sl}_{c}", name=f"g{sl}_{c}")
                 for c in range(CT)] for sl in range(SL)]
        v_sb = [[epool.tile([P, NB], bf, tag=f"v{sl}_{c}", name=f"v{sl}_{c}")
                 for c in range(CT)] for sl in range(SL)]
        ns_sb = [[epool.tile([P, 2, NB], bf, tag=f"ns{sl}_{gidx}",
                             name=f"ns{sl}_{gidx}")
                  for gidx in range(CT // 2)] for sl in range(SL)]

        def bsl(sl):
            return slice(sl * NB, (sl + 1) * NB)

        # ---- input DMA stream (SP queue), in consumption order ----
        def drbr(dram, o, nkb, cols):
            # k-blocks [o, o+nkb) of dram -> [128, nkb*2, n], one DMA
            return dram[o * 256 : (o + nkb) * 256, cols].rearrange(
                "(k s p) b -> p (k s) b", p=P, s=2
            )

        # Gate(s0) inputs lead the stream: all 8 PSUM banks are free at t=0,
        # so the gate matmuls soak up the PE while the big state weight
        # stream is still arriving.
        if gate_mode == "fp8":
            nc.sync.dma_start(
                wg_sb[:, :, 0:512], drbr(wg8d, 0, KB_I, slice(0, 512))
            )
        else:
            for k in range(KT_G):
                nc.sync.dma_start(wg_sb[k][:, 0:256],
                                  wgbd[k * P : (k + 1) * P, 0:256])
        nc.sync.dma_start(xm[0][0][:], dr3(x8d, 0, bsl(0)))
        nc.sync.dma_start(
            xma[0][:, 2:, :],
            x8d[256:D_IN, bsl(0)].rearrange("(k s p) b -> p (k s) b", p=P, s=2),
        )
        if gate_mode == "fp8":
            nc.sync.dma_start(
                wg_sb[:, :, 512:C], drbr(wg8d, 0, KB_I, slice(512, C))
            )
        else:
            for k in range(KT_G):
                nc.sync.dma_start(wg_sb[k][:, 256:C],
                                  wgbd[k * P : (k + 1) * P, 256:C])
                nc.sync.dma_start(xbm[0][k][:], xbd[k * P : (k + 1) * P, bsl(0)])
        nc.sync.dma_start(
            xma[1][:],
            x8d[:, bsl(1)].rearrange("(k s p) b -> p (k s) b", p=P, s=2),
        )
        if gate_mode != "fp8":
            for k in range(KT_G):
                nc.sync.dma_start(xbm[1][k][:], xbd[k * P : (k + 1) * P, bsl(1)])
        for j in range(KB_I // 2):
            nc.sync.dma_start(wi_sb[j][:], drbr(wi8d, 2 * j, 2, slice(0, C)))
        for j, n in enumerate(WRBS):
            nc.sync.dma_start(wr_sb[j][:], drbr(wr8d, WRO[j], n, slice(0, C)))
            nc.sync.dma_start(sm0[j][:], drbr(s8d, WRO[j], n, bsl(0)))
        for j, n in enumerate(S1BS):
            nc.sync.dma_start(sm1[j][:], drbr(s8d, S1O[j], n, bsl(1)))
        for sl in range(SL):
            nc.sync.dma_start(
                sw_sb[sl][:],
                sw9d[:, bsl(sl)].rearrange("(c p) b -> p c b", p=P),
            )

        def state_x_matmuls(st, sl, cts, kbs=None):
            for k in kbs if kbs is not None else range(KB_I):
                j, kk = divmod(k, 2)
                for c in cts:
                    nc.tensor.matmul(
                        st[c][:],
                        wi_sb[j][:, 2 * kk : 2 * kk + 2, c * P : (c + 1) * P],
                        xm[sl][k][:],
                        start=(k == 0),
                        stop=False,
                        perf_mode=DR,
                    )

        def state_s_matmuls(st, sl, cts, kbs):
            for k in kbs:
                for c in cts:
                    nc.tensor.matmul(
                        st[c][:],
                        wr_slice(k)[:, :, c * P : (c + 1) * P],
                        s_slice(sl, k),
                        start=False,
                        stop=(k == KB_R - 1),
                        perf_mode=DR,
                    )

        def gate_matmuls(gt, sl, c):
            cs = slice(c * P, (c + 1) * P)
            if gate_mode == "fp8":
                for k in range(KB_I):
                    nc.tensor.matmul(
                        gt[:], wg_sb[:, 2 * k : 2 * k + 2, cs],
                        xm[sl][k][:],
                        start=(k == 0), stop=(k == KB_I - 1), perf_mode=DR,
                    )
            else:
                for k in range(KT_G):
                    nc.tensor.matmul(
                        gt[:], wg_sb[k][:, cs], xbm[sl][k][:],
                        start=(k == 0), stop=(k == KT_G - 1),
                    )

        gate_scale = 1.0 / WSCALE if gate_mode == "fp8" else 1.0

        def emit_ns(sl, c, halves=1):
            # o = v * g into the ns staging tile, then DMA the column tile
            # out.  halves=2 pipelines the last column's epilogue in two
            # 256-wide chunks so the end-of-kernel drain chain is shorter.
            for h in range(halves):
                hs = slice(h * NB // halves, (h + 1) * NB // halves)
                nc.vector.tensor_tensor(
                    ns_sb[sl][c // 2][:, c % 2, hs], v_sb[sl][c][:, hs],
                    g_sb[sl][c][:, hs], ALU.mult,
                )
                nc.sync.dma_start(
                    nspd[c * P : (c + 1) * P, bsl(sl)][:, hs],
                    ns_sb[sl][c // 2][:, c % 2, hs],
                )

        def emit_sig(sl, c, gt, halves=1):
            for h in range(halves):
                hs = slice(h * NB // halves, (h + 1) * NB // halves)
                nc.scalar.activation(g_sb[sl][c][:, hs], gt[:, hs],
                                     AF.Sigmoid, scale=gate_scale)

        # ---- phase 0: PE warm-up.  The tensor engine clock ramps over its
        # first ~3us of continuous work; zero-matmuls during the DMA
        # lead-in get the ramp done before any real matmul issues.
        zm = mp.tile([P, 2, NB], f8, tag="zwarm", name="zm")
        nc.gpsimd.memset(zm[:], 0)
        warm_ps = pp.tile([P, NB], f32, tag="acc", name="warm_ps")
        for _ in range(16):
            nc.tensor.matmul(warm_ps[:], zm[:, :, 0:P], zm[:],
                             start=True, stop=True, perf_mode=DR)

        # ---- phase 1: BOTH slices' gates, while the state weight stream
        # arrives.  All PSUM banks are free at t=0 and the gates need only
        # wg+x (2MB); the sigmoids hand each bank onward.  This removes all
        # gate work from the back half of the kernel.
        for sl in range(SL):
            for c in range(CT):
                gt_c = pp.tile([P, NB], f32, tag="acc", name=f"gt{sl}_{c}")
                gate_matmuls(gt_c, sl, c)
                emit_sig(sl, c, gt_c)

        # ---- phase 2: slice 0 state, kb-major (DMA-streamed) ----
        st0 = [pp.tile([P, NB], f32, tag="acc", name=f"st0_{c}")
               for c in range(CT)]
        state_x_matmuls(st0, 0, range(CT))
        state_s_matmuls(st0, 0, range(CT), range(KB_R))
        for c in range(CT):
            nc.scalar.activation(t_sb[0][c][:], st0[c][:], AF.Tanh,
                                 scale=1.0 / WSCALE)
        for c in range(CT):
            nc.vector.tensor_tensor(v_sb[0][c][:], t_sb[0][c][:],
                                    sw_sb[0][:, c, :], ALU.add)
            emit_ns(0, c)

        # ---- phases 3/4: slice 1 in two ct-groups of 4.  Within a group:
        # pass A accumulates k-blocks 0..7 (needs only the first half of the
        # s1 stream), then pass B finishes k-blocks 8..15 with the gate
        # matmuls pipelined between columns.  Gate banks come from the
        # OTHER half of the PSUM pool (freed by slice-0 sigmoids / earlier
        # group), so the final gate never waits on the final tanh.
        # ---- phase 3: slice 1 state pass A (k-blocks 0..7), ct-major —
        # needs only the first half of the s1 stream.  Banks come from the
        # slice-0 tanh stream. ----
        KH = KB_R // 2
        st1 = [None] * CT
        for c in range(CT):
            st1[c] = pp.tile([P, NB], f32, tag="acc", name=f"st1_{c}")
            state_x_matmuls(st1, 1, [c])
            state_s_matmuls(st1, 1, [c], range(KH))

        # ---- phase 4: pass B (k-blocks 8..15) + epilogue per column.
        # Gates already done, so the tail is only the final tanh+v+ns. ----
        for c in range(CT):
            last = (c == CT - 1)
            state_s_matmuls(st1, 1, [c], range(KH, KB_R))
            if not last:
                nc.scalar.activation(t_sb[1][c][:], st1[c][:], AF.Tanh,
                                     scale=1.0 / WSCALE)
                nc.vector.tensor_tensor(v_sb[1][c][:], t_sb[1][c][:],
                                        sw_sb[1][:, c, :], ALU.add)
                emit_ns(1, c)
            else:
                # final column: per-half tanh+v+ns+DMA chains so the drain
                # pipeline overlaps the second half's compute
                nc.scalar.activation(t_sb[1][c][:], st1[c][:], AF.Tanh,
                                     scale=1.0 / WSCALE)
                nc.vector.tensor_tensor(v_sb[1][c][:], t_sb[1][c][:],
                                        sw_sb[1][:, c, :], ALU.add)
                emit_ns(1, c)

    nc.compile()
    return nc


def _get_program():
    if GATE_MODE not in _CACHE:
        _CACHE[GATE_MODE] = _build(GATE_MODE)
    return _CACHE[GATE_MODE]


def kernel(inputs, prev_output, reservoir_state, input_weights,
           reservoir_weights, gate_weights):
    from concourse.bass_utils import run_bass_kernel_spmd

    nc = _get_program()

    x = np.ascontiguousarray(np.asarray(inputs, dtype=np.float32))
    s = np.ascontiguousarray(np.asarray(reservoir_state, dtype=np.float32))
    w_in = np.asarray(input_weights, dtype=np.float32)
    w_res = np.asarray(reservoir_weights, dtype=np.float32)
    w_gate = np.asarray(gate_weights, dtype=np.float32)

    xT = np.ascontiguousarray(x.T)           # [D_IN, B]
    sT = np.ascontiguousarray(s.T)           # [R, B]
    x8 = xT.astype(F8NP)
    s8 = sT.astype(F8NP)
    sw9 = (9.0 * sT).astype(BFNP)            # [R, B]
    wi8 = (w_in * WSCALE).astype(F8NP)       # [D_IN, R]
    wr8 = (w_res * WSCALE).astype(F8NP)      # [R, R]
    if GATE_MODE == "fp8":
        wg8 = (w_gate * WSCALE).astype(F8NP)
    else:
        wgb = w_gate.astype(BFNP)
        xb = xT.astype(BFNP)

    in_maps = []
    for core in range(N_CORES):
        bg, cg = divmod(core, CGRP)
        cs = slice(cg * C, (cg + 1) * C)
        bs_ = slice(bg * BS, (bg + 1) * BS)
        m = {
            "x8": np.ascontiguousarray(x8[:, bs_]),
            "s8": np.ascontiguousarray(s8[:, bs_]),
            "sw9": np.ascontiguousarray(sw9[cs, bs_]),
            "wi8": np.ascontiguousarray(wi8[:, cs]),
            "wr8": np.ascontiguousarray(wr8[:, cs]),
        }
        if GATE_MODE == "fp8":
            m["wg8"] = np.ascontiguousarray(wg8[:, cs])
        else:
            m["wgb"] = np.ascontiguousarray(wgb[:, cs])
            m["xb"] = np.ascontiguousarray(xb[:, bs_])
        in_maps.append(m)

    res = run_bass_kernel_spmd(nc, in_maps, list(range(N_CORES)))

    nsp = np.empty((R, B), dtype=np.float32)  # 10 * new_state, transposed
    for core in range(N_CORES):
        bg, cg = divmod(core, CGRP)
        nsp[cg * C : (cg + 1) * C, bg * BS : (bg + 1) * BS] = (
            res.results[core]["nsp"].astype(np.float32)
        )
    new_state = np.ascontiguousarray(nsp.T) * np.float32(0.1)  # [B, R]

    # Re-evaluate borderline elements (|ns - 0.5| < band) in full precision
    # so fp8/bf16 quantization error cannot flip spikes at the threshold.
    bi, rj = np.nonzero(np.abs(new_state - 0.5) < PATCH_BAND)
    if bi.size:
        CH = 32768
        for lo in range(0, bi.size, CH):
            bc, rc = bi[lo : lo + CH], rj[lo : lo + CH]
            xb_ = x[bc]                       # [n, D_IN]
            sb_ = s[bc]                       # [n, R]
            acc = np.einsum("ij,ji->i", xb_, w_in[:, rc], optimize=True)
            acc += np.einsum("ij,ji->i", sb_, w_res[:, rc], optimize=True)
            z = np.einsum("ij,ji->i", xb_, w_gate[:, rc], optimize=True)
            gate = 1.0 / (1.0 + np.exp(-z.astype(np.float64)))
            ns_fix = (0.9 * sb_[np.arange(bc.size), rc].astype(np.float64)
                      + 0.1 * np.tanh(acc.astype(np.float64))) * gate
            new_state[bc, rc] = ns_fix.astype(np.float32)

    output = (new_state > np.float32(0.5)).astype(np.float32)
    return output, new_state
